# revision 2
# baseline (speedup 1.0000x reference)
"""BracketNet Trainium2 kernel.

Problem: per-head bracket scan over seq_len.
  src: [S=2048, B=64, D=512], H=8 heads, DIM=64.
  ctx_t = gelu(W_h @ [ctx_{t-1}; x_t] + b_h)  (per head, exact erf gelu)
  r_t   = x_t + ctx_t

Sharding: one head per NeuronCore (8 cores). Each core's weights
(W_h^T [128,64]) are tiny and stay in SBUF; the 2048-step sequential
scan runs locally with the full batch B=64.

Layout: everything on-chip is "transposed" (head-dim on partitions,
batch on the free axis) so the scan matmul needs no on-chip transposes:
  cat^T tile [128, *]: rows 0:64 = ctx^T (written by ACT gelu),
                       rows 64:128 = x^T (DMA'd from HBM).
  y^T = W_h^T.T @ cat^T   -> PSUM [64, BG]
The host pre-transposes src (pure data movement) into per-head
x^T arrays [DIM, G*S*BG] grouped by batch-half so each core streams
contiguous rows; outputs come back as r^T and are inverse-transposed.

The scan chain (matmul -> gelu -> matmul) is latency-bound, so the
batch is split into G=2 independent groups whose chains interleave on
the PE/ACT engines.
"""

import numpy as np

S, B, D, H = 2048, 64, 512, 8
DIM = 64
G = 2            # independent batch groups per core (pipelined chains)
BG = B // G      # batch columns per group
CH = 64          # timesteps per streamed chunk
NCH = S // CH

_last_run_info = {}


def _build_nc():
    import concourse.mybir as mybir
    from concourse import tile, bacc

    f32 = mybir.dt.float32
    nc = bacc.Bacc("TRN2", target_bir_lowering=False, debug=False)

    xt_ext = nc.declare_dram_parameter("xt", [DIM, G * S * BG], f32, isOutput=False)
    wt_ext = nc.declare_dram_parameter("wt", [2 * DIM, DIM], f32, isOutput=False)
    bias_ext = nc.declare_dram_parameter("bias", [DIM, 1], f32, isOutput=False)
    rt_ext = nc.declare_dram_parameter("rt", [DIM, G * S * BG], f32, isOutput=True)

    gelu = mybir.ActivationFunctionType.Gelu

    with tile.TileContext(nc) as tc:
        with (
            tc.tile_pool(name="const", bufs=1) as cpool,
            tc.tile_pool(name="cat", bufs=3) as catpool,
            tc.tile_pool(name="rst", bufs=2) as rpool,
            tc.tile_pool(name="ps", bufs=3, space="PSUM") as ppool,
        ):
            wt = cpool.tile([2 * DIM, DIM], f32, tag="wt", name="wt")
            nc.sync.dma_start(out=wt[:], in_=wt_ext[:])
            bias = cpool.tile([DIM, 1], f32, tag="bias", name="bias")
            nc.sync.dma_start(out=bias[:], in_=bias_ext[:])

            def new_cat(g):
                return catpool.tile([2 * DIM, CH * BG], f32, tag=f"cat{g}", name=f"cat{g}")

            def dma_x(dest_tiles, c):
                for g in range(G):
                    lo = (g * S + c * CH) * BG
                    hi = (g * S + (c + 1) * CH) * BG
                    nc.gpsimd.dma_start(
                        out=dest_tiles[g][DIM : 2 * DIM, :], in_=xt_ext[:, lo:hi]
                    )

            cat = [new_cat(g) for g in range(G)]
            dma_x(cat, 0)
            for g in range(G):
                nc.vector.memset(cat[g][0:DIM, 0:BG], 0.0)  # ctx_{-1} = 0

            for c in range(NCH):
                cat_next = [new_cat(g) for g in range(G)]
                if c + 1 < NCH:
                    dma_x(cat_next, c + 1)
                r = [rpool.tile([DIM, CH * BG], f32, tag=f"r{g}", name=f"r{g}") for g in range(G)]
                for i in range(CH):
                    for g in range(G):
                        sl = slice(i * BG, (i + 1) * BG)
                        ps = ppool.tile([DIM, BG], f32, tag=f"y{g}", name=f"y{g}")
                        nc.tensor.matmul(
                            ps[:], wt[:], cat[g][:, sl], start=True, stop=True
                        )
                        if i + 1 < CH:
                            dest = cat[g][0:DIM, (i + 1) * BG : (i + 2) * BG]
                        else:
                            dest = cat_next[g][0:DIM, 0:BG]
                        nc.scalar.activation(dest, ps[:], gelu, bias=bias[:])
                        nc.vector.tensor_add(
                            r[g][:, sl], cat[g][DIM : 2 * DIM, sl], dest
                        )
                for g in range(G):
                    lo = (g * S + c * CH) * BG
                    hi = (g * S + (c + 1) * CH) * BG
                    nc.sync.dma_start(out=rt_ext[:, lo:hi], in_=r[g][:])
                cat = cat_next

    nc.compile()
    return nc


_nc_cache = None


def _get_nc():
    global _nc_cache
    if _nc_cache is None:
        _nc_cache = _build_nc()
    return _nc_cache


def kernel(src: np.ndarray, W: np.ndarray, b: np.ndarray) -> np.ndarray:
    import os
    from concourse.bass_utils import run_bass_kernel_spmd

    src = np.ascontiguousarray(src, dtype=np.float32)
    W = np.asarray(W, dtype=np.float32)
    b = np.asarray(b, dtype=np.float32)

    nc = _get_nc()

    in_maps = []
    for h in range(H):
        xh = src[:, :, h * DIM : (h + 1) * DIM]          # (t, b, d)
        xh = xh.reshape(S, G, BG, DIM)                   # (t, g, b, d)
        xt = np.ascontiguousarray(xh.transpose(3, 1, 0, 2)).reshape(DIM, G * S * BG)
        in_maps.append(
            {
                "xt": xt,
                "wt": np.ascontiguousarray(W[h].T),      # [2*DIM, DIM]
                "bias": np.ascontiguousarray(b[h].reshape(DIM, 1)),
            }
        )

    trace = bool(os.environ.get("BASS_TRACE"))
    res = run_bass_kernel_spmd(nc, in_maps, list(range(H)), trace=trace)
    _last_run_info["exec_time_ns"] = res.exec_time_ns
    _last_run_info["profile_json"] = res.profile_json

    out = np.empty((S, B, D), dtype=np.float32)
    for h in range(H):
        rt = res.results[h]["rt"].reshape(DIM, G, S, BG)
        out[:, :, h * DIM : (h + 1) * DIM] = (
            rt.transpose(2, 1, 3, 0).reshape(S, B, DIM)
        )
    return out


# revision 3
# speedup vs baseline: 1.3002x; 1.3002x over previous
"""BracketNet Trainium2 kernel, v3: time-sliced scan chains, fused cat matmul.

Same chain/burn-in scheme as v2 (see kernel2.py), but each joint-step is ONE
K=128 matmul over a shared cat^T tile ([128, J*64]: partitions 0:64 = ctx^T
written by the gelu ACT, 64:128 = x^T written by the input DMA), halving PE
work (one LDWEIGHTS+MATMUL instead of two). The r-add reads x^T from a
base-0 copy made once per chunk by a single wide DVE copy (cross-partition
single-input copies are legal; two-input ops require equal base partitions).

Everything 2-byte fp16 (x, ctx, weights, r): full-rate PE, DVE 2x/4x modes,
halved DMA; fp32 PSUM accumulation and fp32 ACT internals keep error ~3e-4.
"""

import numpy as np

S, B, D, H = 2048, 64, 512, 8
DIM = 64

J = 8            # chains per joint group (one ACT instruction covers J)
P = 3            # joint groups (latency-hiding factor)
T = J * P        # total time-sliced chains
L = 114          # steps each chain executes (own + burn-in)
CH = 6           # joint-steps per streamed chunk
MIN_BURN = 24    # minimum burn-in steps for chains 1..T-1

REPS = 1         # repeat the whole body (timing runs only)

JB = J * B       # free width of one joint-step block (J chains x 64 batch)

_last_run_info = {}


def _chain_ends():
    rest = S - L
    base, extra = divmod(rest, T - 1)
    assert base + 1 <= L - MIN_BURN, "burn-in too short; raise L or lower T"
    owns = [L] + [base + 1] * extra + [base] * (T - 1 - extra)
    ends = np.cumsum(owns)
    assert ends[-1] == S
    return [int(e) for e in ends], owns


def _build_nc():
    import concourse.mybir as mybir
    from concourse import tile, bacc

    f32 = mybir.dt.float32
    f16 = mybir.dt.float16
    nc = bacc.Bacc("TRN2", target_bir_lowering=False, debug=False)

    NSTREAM = P * L * JB
    xt_ext = nc.declare_dram_parameter("xt", [DIM, NSTREAM], f16, isOutput=False)
    wt_ext = nc.declare_dram_parameter("wt", [2 * DIM, DIM], f16, isOutput=False)
    bias_ext = nc.declare_dram_parameter("bias", [DIM, 1], f32, isOutput=False)
    rt_ext = nc.declare_dram_parameter("rt", [DIM, NSTREAM], f16, isOutput=True)

    gelu = mybir.ActivationFunctionType.Gelu
    NCH = L // CH
    assert NCH * CH == L

    with tile.TileContext(nc) as tc:
        with (
            tc.tile_pool(name="const", bufs=1) as cpool,
            tc.tile_pool(name="catp", bufs=3) as catpool,
            tc.tile_pool(name="xlp", bufs=2) as xlpool,
            tc.tile_pool(name="rst", bufs=2) as rpool,
            tc.tile_pool(name="ps", bufs=2, space="PSUM") as ppool,
        ):
            wt = cpool.tile([2 * DIM, DIM], f16, tag="wt", name="wt")
            nc.sync.dma_start(out=wt[:], in_=wt_ext[:])
            bias = cpool.tile([DIM, 1], f32, tag="bias", name="bias")
            nc.sync.dma_start(out=bias[:], in_=bias_ext[:])

            def body():
                def new_cat(p):
                    return catpool.tile(
                        [2 * DIM, CH * JB], f16, tag=f"cat{p}", name=f"cat{p}"
                    )

                def dma_x(dest_tiles, c):
                    for p in range(P):
                        lo = (p * L + c * CH) * JB
                        hi = (p * L + (c + 1) * CH) * JB
                        nc.gpsimd.dma_start(
                            out=dest_tiles[p][DIM : 2 * DIM, :],
                            in_=xt_ext[:, lo:hi],
                        )

                cat = [new_cat(p) for p in range(P)]
                dma_x(cat, 0)
                for p in range(P):
                    nc.vector.memset(cat[p][0:DIM, 0:JB], 0.0)  # ctx_{-1} = 0

                for c in range(NCH):
                    cat_next = [new_cat(p) for p in range(P)]
                    if c + 1 < NCH:
                        dma_x(cat_next, c + 1)
                    xlo = [
                        xlpool.tile(
                            [DIM, CH * JB], f16, tag=f"xlo{p}", name=f"xlo{p}"
                        )
                        for p in range(P)
                    ]
                    for p in range(P):
                        # base-0 copy of x^T for the r-add (single wide copy)
                        nc.vector.tensor_copy(xlo[p][:], cat[p][DIM : 2 * DIM, :])
                    r = [
                        rpool.tile([DIM, CH * JB], f16, tag=f"r{p}", name=f"r{p}")
                        for p in range(P)
                    ]
                    for i in range(CH):
                        for p in range(P):
                            sl = slice(i * JB, (i + 1) * JB)
                            ps = ppool.tile(
                                [DIM, JB], f32, tag=f"y{p}", name=f"y{p}"
                            )
                            nc.tensor.matmul(
                                ps[:], wt[:], cat[p][:, sl], start=True, stop=True
                            )
                            if i + 1 < CH:
                                dest = cat[p][0:DIM, (i + 1) * JB : (i + 2) * JB]
                            else:
                                dest = cat_next[p][0:DIM, 0:JB]
                            nc.scalar.activation(dest, ps[:], gelu, bias=bias[:])
                            nc.vector.tensor_add(r[p][:, sl], xlo[p][:, sl], dest)
                    for p in range(P):
                        lo = (p * L + c * CH) * JB
                        hi = (p * L + (c + 1) * CH) * JB
                        nc.sync.dma_start(out=rt_ext[:, lo:hi], in_=r[p][:])
                    cat = cat_next

            if REPS == 1:
                body()
            else:
                with tc.For_i(0, REPS, 1):
                    body()

    nc.compile()
    return nc


_nc_cache = None


def _get_nc():
    global _nc_cache
    if _nc_cache is None:
        _nc_cache = _build_nc()
    return _nc_cache


def _make_in_maps(src, W, b):
    ends, owns = _chain_ends()
    in_maps = []
    for h in range(H):
        xh = src[:, :, h * DIM : (h + 1) * DIM]  # [S, B, DIM] (t, b, d)
        xt = np.empty((DIM, P, L, J, B), dtype=np.float32)
        for k in range(T):
            p, j = divmod(k, J)
            s0 = ends[k] - L
            xt[:, p, :, j, :] = xh[s0 : s0 + L].transpose(2, 0, 1)
        in_maps.append(
            {
                "xt": np.ascontiguousarray(xt)
                .reshape(DIM, P * L * JB)
                .astype(np.float16),
                "wt": np.ascontiguousarray(W[h].T).astype(np.float16),
                "bias": np.ascontiguousarray(b[h].reshape(DIM, 1)),
            }
        )
    return in_maps, ends, owns


def _assemble(results, ends, owns):
    out = np.empty((S, B, D), dtype=np.float32)
    for h in range(H):
        rt = results[h]["rt"].astype(np.float32).reshape(DIM, P, L, J, B)
        for k in range(T):
            p, j = divmod(k, J)
            own = owns[k]
            blk = rt[:, p, L - own :, j, :]  # [DIM, own, B]
            out[ends[k] - own : ends[k], :, h * DIM : (h + 1) * DIM] = (
                blk.transpose(1, 2, 0)
            )
    return out


def kernel(src: np.ndarray, W: np.ndarray, b: np.ndarray) -> np.ndarray:
    import os
    from concourse.bass_utils import run_bass_kernel_spmd

    src = np.ascontiguousarray(src, dtype=np.float32)
    W = np.asarray(W, dtype=np.float32)
    b = np.asarray(b, dtype=np.float32)

    nc = _get_nc()
    in_maps, ends, owns = _make_in_maps(src, W, b)

    trace = bool(os.environ.get("BASS_TRACE"))
    res = run_bass_kernel_spmd(nc, in_maps, list(range(H)), trace=trace)
    _last_run_info["exec_time_ns"] = res.exec_time_ns
    _last_run_info["profile_json"] = res.profile_json

    return _assemble(res.results, ends, owns)
